# revision 12
# baseline (speedup 1.0000x reference)
"""Multi-head attention (B=2, T=2048, D=512, H=8) on 8 trn2 NeuronCores.

Sharding: data + head parallel.  Core c handles batch b = c//4 and head pair
p = c%4 (heads 2p, 2p+1 <-> feature rows 128p .. 128p+127 of the 512-wide
projection space).  Host sums the 4 partial outputs per batch (the
"all-reduce") and adds bo.

Per-core pipeline (v2 -- transposed-PV orientation):
  - project q/k into [feat, tok] (feat on partitions) and v into
    [tok, feat] tiles with a ones column (v_aug),
  - scoresT = k_h q_h^T in [key, query] orientation (keys on partitions),
  - softmax exp on ACT, with a tunable subset of key tiles routed to DVE
    via a Schraudolph int16 bit-trick (bits of round(s*128/ln2 + C)
    reinterpret as bf16 ~= exp(s)), offloading the ACT bottleneck,
  - PV in the TRANSPOSED orientation: out[query, feat] = ex^T @ v_aug,
    queries on psum partitions.  Halves PE cycles vs [feat, query] and
    makes the softmax denominator a per-partition scalar (column 64), so
    normalization is a cheap DVE broadcast multiply,
  - att tiles [q, 2*64] transpose to [feat, q] via the DMA XBAR (no PE
    cycles, no PSUM bank); the final query block uses a PE transpose
    instead to avoid the ~2.5us DMA latency on the drain path,
  - Wo projection: single K=128 matmul per q-tile (both heads fused).
"""

import os
import sys

sys.path.insert(0, "/opt/trn_rl_repo")

from contextlib import ExitStack

import numpy as np
import ml_dtypes

import concourse.bass as bass
import concourse.tile as tile
from concourse import bacc, masks, mybir
from concourse.bass_utils import run_bass_kernel_spmd

BF16 = mybir.dt.bfloat16
F32 = mybir.dt.float32
I16 = mybir.dt.int16

B, T, D = 2, 2048, 512
H, DK = 8, 64
N_CORES = 8
P = 128  # partitions / head-pair feature count
KC = D // P  # 4 contraction chunks of 128 over d_model
NKT = T // P  # 16 key tiles of 128
NQB = 4  # query blocks
QB = T // NQB  # 512 queries per block
QSUB = QB // P  # 4 q-tiles of 128 queries per block
NTC = 4  # token chunks for pipelined loads/projections

# key tiles whose softmax exp runs on DVE via the Schraudolph bit trick
# (per query block).  Empty set = all exp on ACT.
SCHRAUD_KTS = frozenset()


def _build_bass(with_bias):
    nc = bacc.Bacc(trn_type="TRN2", num_devices=N_CORES, debug=False)

    qt_d = nc.dram_tensor("qt", [D, T], BF16, kind="ExternalInput").ap()
    kt_d = nc.dram_tensor("ktin", [D, T], BF16, kind="ExternalInput").ap()
    vt_d = nc.dram_tensor("vt", [D, T], BF16, kind="ExternalInput").ap()
    # q/k/v weights arrive host-pre-swizzled as one [p, 3, c, f]
    # (partition-major) tensor so a single contiguous DMA loads all three
    wqkv_d = nc.dram_tensor("wqkv", [P, 3, KC, P], BF16, kind="ExternalInput").ap()
    # Wo rows for this core's 128 features, [feat, 512] (= Wo.T slice)
    wot_d = nc.dram_tensor("wot", [P, D], BF16, kind="ExternalInput").ap()
    if with_bias:
        bq_d = nc.dram_tensor("bq", [P, 1], F32, kind="ExternalInput").ap()
        bk_d = nc.dram_tensor("bk", [P, 1], F32, kind="ExternalInput").ap()
        bv_d = nc.dram_tensor("bv", [1, P], F32, kind="ExternalInput").ap()
    out_d = nc.dram_tensor("outp", [T, D], F32, kind="ExternalOutput").ap()

    inv_sqrt_dk = float(1.0 / np.sqrt(DK))
    # Schraudolph constants: bits = round(s*inv_sqrt_dk*128/ln2 + EXPC)
    # reinterpreted as bf16.  EXPC centered for ~zero-mean relative error.
    EXPF = float(inv_sqrt_dk * 128.0 / np.log(2.0))
    EXPC = float(16256.0 - 128.0 * 0.0434)

    with tile.TileContext(nc) as tc, ExitStack() as ctx:
        singles = ctx.enter_context(tc.tile_pool(name="singles", bufs=1))
        qk_pool = ctx.enter_context(tc.tile_pool(name="qk", bufs=1))
        v_pool = ctx.enter_context(tc.tile_pool(name="vaug", bufs=NKT))
        exp_pool = ctx.enter_context(tc.tile_pool(name="exps", bufs=4))
        rden_pool = ctx.enter_context(tc.tile_pool(name="rden", bufs=2))
        att_pool = ctx.enter_context(tc.tile_pool(name="att", bufs=5))
        attT_pool = ctx.enter_context(tc.tile_pool(name="attT", bufs=5))
        out_pool = ctx.enter_context(tc.tile_pool(name="outs", bufs=3))
        # PSUM: scores 2x2 banks + pv 2 banks + misc 2x1 banks = 8 banks
        ps_s = ctx.enter_context(tc.tile_pool(name="ps_s", bufs=2, space="PSUM"))
        ps_pv = ctx.enter_context(tc.tile_pool(name="ps_pv", bufs=1, space="PSUM"))
        ps_mi = ctx.enter_context(tc.tile_pool(name="ps_mi", bufs=2, space="PSUM"))

        # ---- weight/bias loads ----
        wqkv_sb = singles.tile([P, 3, KC, P], BF16)
        nc.sync.dma_start(out=wqkv_sb, in_=wqkv_d)
        wqt_sb = wqkv_sb[:, 0]
        wkt_sb = wqkv_sb[:, 1]
        wvt_sb = wqkv_sb[:, 2]
        if with_bias:
            bq_sb = singles.tile([P, 1], F32)
            nc.sync.dma_start(out=bq_sb, in_=bq_d)
            bk_sb = singles.tile([P, 1], F32)
            nc.sync.dma_start(out=bk_sb, in_=bk_d)
            bv_sb = singles.tile([P, P], F32)
            nc.gpsimd.dma_start(
                out=bv_sb,
                in_=bass.AP(tensor=bv_d.tensor, offset=0, ap=[[0, P], [1, P]]),
            )

        # identity for the PE transposes in the final tail
        ident = singles.tile([P, P], BF16)
        masks.make_identity(nc, ident[:, :])
        # zero operand for the PSUM-clearing dummy matmuls (see emit_pv_init)
        zeros = singles.tile([1, D], BF16)
        nc.vector.memset(zeros, 0.0)
        if SCHRAUD_KTS:
            expc_sb = singles.tile([P, 1], F32)
            nc.gpsimd.memset(expc_sb, EXPC)

        # ---- chunked input loads (512-token slices) ----
        qt_sb = singles.tile([P, KC, T], BF16)
        kt_sb = singles.tile([P, KC, T], BF16)
        vt_sb = singles.tile([P, KC, T], BF16)
        # qb0 only needs QT chunk 0; all of KT/VT gate qb0's PV chain,
        # so load those first and defer QT chunks 1-3.
        # First K slice is only 128 tokens (one k-tile) so the first
        # QK->exp fires as soon as possible.
        ktr = kt_d.rearrange("(c p) t -> p c t", p=P)
        qtr = qt_d.rearrange("(c p) t -> p c t", p=P)
        vtr = vt_d.rearrange("(c p) t -> p c t", p=P)
        nc.sync.dma_start(out=kt_sb[:, :, 0:P], in_=ktr[:, :, 0:P])
        nc.sync.dma_start(out=qt_sb[:, :, 0 : T // NTC // 2], in_=qtr[:, :, 0 : T // NTC // 2])
        nc.sync.dma_start(
            out=qt_sb[:, :, T // NTC // 2 : T // NTC],
            in_=qtr[:, :, T // NTC // 2 : T // NTC],
        )
        nc.sync.dma_start(out=kt_sb[:, :, P : T // NTC], in_=ktr[:, :, P : T // NTC])
        # KT chunk c+1 is prefetched ahead of QT/VT chunk c: K gates the
        # QK->exp critical path while V only feeds the lagging PV chain.
        for c in range(1, NTC):
            sl = bass.ts(c, T // NTC)
            nc.sync.dma_start(out=kt_sb[:, :, sl], in_=ktr[:, :, sl])
            slp = bass.ts(c - 1, T // NTC)
            if c >= 2:
                nc.sync.dma_start(out=qt_sb[:, :, slp], in_=qtr[:, :, slp])
            nc.sync.dma_start(out=vt_sb[:, :, slp], in_=vtr[:, :, slp])
        slz = bass.ts(NTC - 1, T // NTC)
        nc.sync.dma_start(out=qt_sb[:, :, slz], in_=qtr[:, :, slz])
        nc.sync.dma_start(out=vt_sb[:, :, slz], in_=vtr[:, :, slz])
        wot_sb = singles.tile([P, D], BF16)
        nc.sync.dma_start(out=wot_sb, in_=wot_d)

        # ---- projections ----
        qT = qk_pool.tile([P, T], BF16)
        kT = qk_pool.tile([P, T], BF16)
        v_aug = [None] * NKT

        bqs = bq_sb if with_bias else None
        bks = bk_sb if with_bias else None

        def emit_qk_proj(dst, src_sb, w_sb, b_sb, c, lo=None, cs=None):
            if cs is None:
                cs = T // NTC
            sl = bass.ds(c * (T // NTC) if lo is None else lo, cs)
            psq = ps_mi.tile([P, QB], F32, tag="mi")
            for kc in range(KC):
                nc.tensor.matmul(
                    psq[:, 0:cs],
                    w_sb[:, kc, :],
                    src_sb[:, kc, sl],
                    start=(kc == 0),
                    stop=(kc == KC - 1),
                )
            nc.vector.tensor_copy(dst[:, sl], psq[:, 0:cs])
            if b_sb is not None:
                nc.vector.tensor_add(
                    dst[:, sl], dst[:, sl], b_sb[:, :].broadcast_to([P, cs])
                )

        def emit_v_proj(kt):
            psv = ps_mi.tile([P, P], F32, tag="mi")
            for kc in range(KC):
                nc.tensor.matmul(
                    psv,
                    vt_sb[:, kc, bass.ts(kt, P)],
                    wvt_sb[:, kc, :],
                    start=(kc == 0),
                    stop=(kc == KC - 1),
                )
            va = v_pool.tile([P, 2, DK + 1], BF16, tag="vaug")
            nc.vector.tensor_copy(
                va[:, :, 0:DK], psv[:, :].rearrange("p (h f) -> p h f", h=2)
            )
            if with_bias:
                nc.vector.tensor_add(
                    va[:, :, 0:DK],
                    va[:, :, 0:DK],
                    bv_sb[:, :].rearrange("p (h f) -> p h f", h=2),
                )
            nc.vector.memset(va[:, :, DK : DK + 1], 1.0)
            v_aug[kt] = va

        # ---- attention ----
        # PSUM rule: a matmul accumulation `start` clears the has_written
        # bits of the WHOLE bank, so the 8 interleaved pv accumulation
        # groups (2 heads x 4 q-tiles, one bank per head) cannot each issue
        # their own start.  Instead one dummy K=1 matmul of zeros per bank
        # writes the full bank (setting every has_written bit); all pv
        # matmuls then accumulate with start=False.
        def emit_pv_init(pv):
            for h in range(2):
                nc.tensor.matmul(
                    pv[:, h].rearrange("p q f -> p (q f)"),
                    zeros[0:1, 0:P],
                    zeros[0:1, :],
                    start=True,
                    stop=True,
                )

        def emit_pv(pv, kt, ex):
            # transposed PV: out[q, dk+1] accumulates over key tiles
            for h in range(2):
                for qs in range(QSUB):
                    nc.tensor.matmul(
                        pv[:, h, qs, 0 : DK + 1],
                        ex[:, h, bass.ts(qs, P)],
                        v_aug[kt][:, h, :],
                        start=False,
                        stop=(kt == NKT - 1),
                        skip_group_check=True,
                    )

        def emit_attn_group(qb, pv, kts, pending, lag=True):
            # software-pipelined: PV(kt) is emitted AFTER QK/exp(kt+1) so the
            # exp engines are never starved of fresh scores.
            for kt in kts:
                sc = ps_s.tile([P, 2, QB], F32, tag="sc")
                for h in range(2):
                    nc.tensor.matmul(
                        sc[:, h, :],
                        kT[DK * h : DK * (h + 1), bass.ts(kt, P)],
                        qT[DK * h : DK * (h + 1), bass.ts(qb, QB)],
                        start=True,
                        stop=True,
                    )
                if kt in SCHRAUD_KTS:
                    exi = exp_pool.tile([P, 2, QB], I16, tag="exps")
                    nc.vector.scalar_tensor_tensor(
                        out=exi,
                        in0=sc,
                        scalar=EXPF,
                        in1=expc_sb[:, :].broadcast_to([P, 2, QB]),
                        op0=mybir.AluOpType.mult,
                        op1=mybir.AluOpType.add,
                    )
                    ex = exi.bitcast(BF16)
                else:
                    ex = exp_pool.tile([P, 2, QB], BF16, tag="exps")
                    nc.scalar.activation(
                        out=ex,
                        in_=sc,
                        func=mybir.ActivationFunctionType.Exp,
                        scale=inv_sqrt_dk,
                    )
                if len(pending) >= 2:
                    emit_pv(*pending.pop(0))
                if lag:
                    pending.append((pv, kt, ex))
                else:
                    emit_pv(pv, kt, ex)

        def emit_tail_front(qb, pv, last):
            # normalize att[q, h, f] = pv[q, h, f] / pv[q, h, 64] and kick
            # off the att -> [feat, q] transposes (DMA XBAR mid-kernel, PE
            # transpose on the drain-critical final block).
            rall = rden_pool.tile([P, 2, QSUB], F32, tag="rden")
            nc.vector.reciprocal(rall, pv[:, :, :, DK])
            atts = []
            for qs in range(QSUB):
                att = att_pool.tile([P, 2, DK], BF16, tag="att")
                for h in range(2):
                    nc.vector.tensor_mul(
                        att[:, h, :],
                        pv[:, h, qs, 0:DK],
                        rall[:, h, qs : qs + 1].broadcast_to([P, DK]),
                    )
                attT = attT_pool.tile([P, P], BF16, tag="attT")
                if last:
                    trp = ps_s.tile([P, P], BF16, tag="sc")
                    nc.tensor.matmul(
                        trp,
                        att[:, :, :].rearrange("p h f -> p (h f)"),
                        ident,
                        is_transpose=True,
                    )
                    nc.vector.tensor_copy(attT, trp)
                else:
                    nc.sync.dma_start(
                        out=attT,
                        in_=att[:, :, :].rearrange("p h f -> p (h f)"),
                        transpose=True,
                    )
                atts.append(attT)
            return atts

        def emit_tail_back(qb, atts, last):
            for qs, attT in enumerate(atts):
                pso = ps_mi.tile([P, D], F32, tag="mi")
                nc.tensor.matmul(pso, attT, wot_sb, start=True, stop=True)
                out_sb = out_pool.tile([P, D], F32, tag="outs")
                nc.vector.tensor_copy(out_sb, pso)
                # out DMAs go on queues whose sequencers have no later
                # compute to issue; the final block uses SP+ACT (lowest
                # issue latency) to shorten the drain.
                if last:
                    eng = nc.sync if qs % 2 == 0 else nc.scalar
                else:
                    eng = nc.sync if qs % 2 == 0 else nc.gpsimd
                eng.dma_start(out=out_d[bass.ds(qb * QB + qs * P, P), :], in_=out_sb)

        # qb0 is interleaved with the per-chunk projections so the exp
        # engines (the bottleneck) start as early as possible.
        pending = []
        pv0 = ps_pv.tile([P, 2, QSUB, P], F32, tag="pv")
        emit_pv_init(pv0)
        emit_qk_proj(kT, kt_sb, wkt_sb, bks, 0, lo=0, cs=P)
        emit_qk_proj(qT, qt_sb, wqt_sb, bqs, 0, lo=0, cs=T // NTC // 2)
        emit_qk_proj(qT, qt_sb, wqt_sb, bqs, 0, lo=T // NTC // 2, cs=T // NTC // 2)
        emit_attn_group(0, pv0, [0], pending)
        emit_qk_proj(kT, kt_sb, wkt_sb, bks, 0, lo=P, cs=T // NTC - P)
        for c in range(NTC):
            # next chunk's k/q projections are emitted BEFORE this chunk's
            # v-projections so their psum allocations are FIFO-ahead of the
            # (VT-gated) psv tiles in the shared misc pool.
            if c + 1 < NTC:
                emit_qk_proj(kT, kt_sb, wkt_sb, bks, c + 1)
            if c == 0:
                for kt in range(4):
                    emit_v_proj(kt)
            lo = 1 if c == 0 else 4 * c
            emit_attn_group(0, pv0, range(lo, 4 * (c + 1)), pending)
            if c + 1 < NTC:
                for kt in range(4 * (c + 1), 4 * (c + 2)):
                    emit_v_proj(kt)
            if c + 1 < NTC:
                emit_qk_proj(qT, qt_sb, wqt_sb, bqs, c + 1)
        # Tails are split: the front (normalize + transpose) is emitted
        # early in the NEXT qb so the XBAR transposes complete before the
        # back (Wo + copies + DMA) needs them mid-qb -- no PE bubbles.
        prev = (0, pv0)
        for qb in range(1, NQB):
            pv = ps_pv.tile([P, 2, QSUB, P], F32, tag="pv")
            emit_pv_init(pv)
            last = qb == NQB - 1
            emit_attn_group(qb, pv, range(0, 2), pending)
            atts = emit_tail_front(*prev, last=False)
            emit_attn_group(qb, pv, range(2, 6), pending)
            emit_tail_back(prev[0], atts, last=False)
            emit_attn_group(qb, pv, range(6, NKT), pending)
            if last:
                while pending:
                    emit_pv(*pending.pop(0))
            prev = (qb, pv)
        atts = emit_tail_front(*prev, last=True)
        emit_tail_back(prev[0], atts, last=True)

    nc.compile()
    return nc


_NC_CACHE = {}


def _get_nc(with_bias):
    if with_bias not in _NC_CACHE:
        _NC_CACHE[with_bias] = _build_bass(with_bias)
    return _NC_CACHE[with_bias]


def _prep_in_maps(Q, K, V, Wq, bq, Wk, bk, Wv, bv, Wo, bo, with_bias):
    bf = ml_dtypes.bfloat16
    f32 = np.float32
    qkvT = []  # per batch: transposed bf16 [D, T]
    for X in (Q, K, V):
        qkvT.append([np.ascontiguousarray(X[b].T.astype(bf)) for b in range(B)])
    woT = np.ascontiguousarray(Wo.T.astype(bf))  # [D feat, D out]

    def swz(w_rows):  # [P, D] slice of W -> transposed+partition-major [P, KC, P]
        return np.ascontiguousarray(
            w_rows.T.astype(bf).reshape(KC, P, P).transpose(1, 0, 2)
        )

    in_maps = []
    for c in range(N_CORES):
        b, p = divmod(c, 4)
        rows = slice(P * p, P * (p + 1))
        m = {
            "qt": qkvT[0][b],
            "ktin": qkvT[1][b],
            "vt": qkvT[2][b],
            "wqkv": np.ascontiguousarray(
                np.stack([swz(Wq[rows]), swz(Wk[rows]), swz(Wv[rows])], axis=1)
            ),
            "wot": np.ascontiguousarray(woT[rows]),
        }
        if with_bias:
            m["bq"] = np.ascontiguousarray(bq[rows].astype(f32).reshape(P, 1))
            m["bk"] = np.ascontiguousarray(bk[rows].astype(f32).reshape(P, 1))
            m["bv"] = np.ascontiguousarray(bv[rows].astype(f32).reshape(1, P))
        in_maps.append(m)
    return in_maps


def kernel(Q, K, V, Wq, bq, Wk, bk, Wv, bv, Wo, bo, _return_raw=False):
    # accept jax arrays / lists transparently
    Q, K, V = np.asarray(Q), np.asarray(K), np.asarray(V)
    Wq, Wk, Wv, Wo = (np.asarray(x) for x in (Wq, Wk, Wv, Wo))
    bq, bk, bv, bo = (np.asarray(x) for x in (bq, bk, bv, bo))
    with_bias = bool(np.any(bq) or np.any(bk) or np.any(bv))
    nc = _get_nc(with_bias)
    in_maps = _prep_in_maps(Q, K, V, Wq, bq, Wk, bk, Wv, bv, Wo, bo, with_bias)
    try:
        res = run_bass_kernel_spmd(
            nc,
            in_maps,
            core_ids=list(range(N_CORES)),
            trace=os.environ.get("KERNEL_TRACE", "0") == "1",
        )
    except ModuleNotFoundError:
        # BASS_TRACE was requested but this axon build lacks the NTFF
        # profile hook (antenv.axon_hooks) -- rerun with tracing disabled.
        os.environ["BASS_NEVER_TRACE"] = "1"
        res = run_bass_kernel_spmd(
            nc, in_maps, core_ids=list(range(N_CORES)), trace=False
        )
    parts = [r["outp"] for r in res.results]
    out = np.empty((B, T, D), np.float32)
    for b in range(B):
        out[b] = parts[4 * b]
        for p in range(1, 4):
            out[b] += parts[4 * b + p]
        out[b] += bo.astype(np.float32)
    if _return_raw:
        return out, res
    return out


# revision 34
# speedup vs baseline: 1.0149x; 1.0149x over previous
"""Multi-head attention (B=2, T=2048, D=512, H=8) on 8 trn2 NeuronCores.

Sharding: data + head parallel.  Core c handles batch b = c//4 and head pair
p = c%4 (heads 2p, 2p+1 <-> feature rows 128p .. 128p+127 of the 512-wide
projection space).  Host sums the 4 partial outputs per batch (the
"all-reduce") and adds bo.

Per-core pipeline (v2 -- transposed-PV orientation):
  - project q/k into [feat, tok] (feat on partitions) and v into
    [tok, feat] tiles with a ones column (v_aug),
  - scoresT = k_h q_h^T in [key, query] orientation (keys on partitions),
  - softmax exp on ACT, with a tunable subset of key tiles routed to DVE
    via a Schraudolph int16 bit-trick (bits of round(s*128/ln2 + C)
    reinterpret as bf16 ~= exp(s)), offloading the ACT bottleneck,
  - PV in the TRANSPOSED orientation: out[query, feat] = ex^T @ v_aug,
    queries on psum partitions.  Halves PE cycles vs [feat, query] and
    makes the softmax denominator a per-partition scalar (column 64), so
    normalization is a cheap DVE broadcast multiply,
  - att tiles [q, 2*64] transpose to [feat, q] via the DMA XBAR (no PE
    cycles, no PSUM bank); the final query block uses a PE transpose
    instead to avoid the ~2.5us DMA latency on the drain path,
  - Wo projection: single K=128 matmul per q-tile (both heads fused).
"""

import os
import sys

sys.path.insert(0, "/opt/trn_rl_repo")

from contextlib import ExitStack

import numpy as np
import ml_dtypes

import concourse.bass as bass
import concourse.tile as tile
from concourse import bacc, masks, mybir
from concourse.bass_utils import run_bass_kernel_spmd

BF16 = mybir.dt.bfloat16
F32 = mybir.dt.float32
I16 = mybir.dt.int16

B, T, D = 2, 2048, 512
H, DK = 8, 64
N_CORES = 8
P = 128  # partitions / head-pair feature count
KC = D // P  # 4 contraction chunks of 128 over d_model
NKT = T // P  # 16 key tiles of 128
NQB = 4  # query blocks
QB = T // NQB  # 512 queries per block
QSUB = QB // P  # 4 q-tiles of 128 queries per block
NTC = 4  # token chunks for pipelined loads/projections

# (qb, kt) pairs whose softmax exp runs on DVE via the Schraudolph bit
# trick, offloading the ACT bottleneck.  qb0 is PE/DMA-paced (projections)
# so ACT keeps all 16 tiles there; qb1-3 each offload 5.
SCHRAUD_KTS = frozenset(
    (qb, kt) for qb in (1, 2, 3) for kt in (2, 5, 8, 11, 14)
)


def _build_bass(with_bias):
    nc = bacc.Bacc(trn_type="TRN2", num_devices=N_CORES, debug=False)

    qt_d = nc.dram_tensor("qt", [D, T], BF16, kind="ExternalInput").ap()
    kt_d = nc.dram_tensor("ktin", [D, T], BF16, kind="ExternalInput").ap()
    vt_d = nc.dram_tensor("vt", [D, T], BF16, kind="ExternalInput").ap()
    # q/k/v weights arrive host-pre-swizzled as one [p, 3, c, f]
    # (partition-major) tensor so a single contiguous DMA loads all three
    wqkv_d = nc.dram_tensor("wqkv", [P, 3, KC, P], BF16, kind="ExternalInput").ap()
    # Wo rows for this core's 128 features, [feat, 512] (= Wo.T slice)
    wot_d = nc.dram_tensor("wot", [P, D], BF16, kind="ExternalInput").ap()
    if with_bias:
        bq_d = nc.dram_tensor("bq", [P, 1], F32, kind="ExternalInput").ap()
        bk_d = nc.dram_tensor("bk", [P, 1], F32, kind="ExternalInput").ap()
        bv_d = nc.dram_tensor("bv", [1, P], F32, kind="ExternalInput").ap()
    out_d = nc.dram_tensor("outp", [T, D], F32, kind="ExternalOutput").ap()

    inv_sqrt_dk = float(1.0 / np.sqrt(DK))
    # Schraudolph constants: bits = round(s*inv_sqrt_dk*128/ln2 + EXPC)
    # reinterpreted as bf16.  EXPC centered for ~zero-mean relative error.
    EXPF = float(inv_sqrt_dk * 128.0 / np.log(2.0))
    EXPC = float(16256.0 - 128.0 * 0.0579)  # zero-mean relative error

    with tile.TileContext(nc) as tc, ExitStack() as ctx:
        singles = ctx.enter_context(tc.tile_pool(name="singles", bufs=1))
        qk_pool = ctx.enter_context(tc.tile_pool(name="qk", bufs=1))
        v_pool = ctx.enter_context(tc.tile_pool(name="vaug", bufs=NKT))
        exp_pool = ctx.enter_context(tc.tile_pool(name="exps", bufs=4))
        rden_pool = ctx.enter_context(tc.tile_pool(name="rden", bufs=2))
        att_pool = ctx.enter_context(tc.tile_pool(name="att", bufs=5))
        attT_pool = ctx.enter_context(tc.tile_pool(name="attT", bufs=5))
        out_pool = ctx.enter_context(tc.tile_pool(name="outs", bufs=3))
        # PSUM: scores 2x2 banks + pv 2 banks + misc 2x1 banks = 8 banks
        ps_s = ctx.enter_context(tc.tile_pool(name="ps_s", bufs=2, space="PSUM"))
        ps_pv = ctx.enter_context(tc.tile_pool(name="ps_pv", bufs=1, space="PSUM"))
        ps_mi = ctx.enter_context(tc.tile_pool(name="ps_mi", bufs=2, space="PSUM"))

        # ---- weight/bias loads ----
        # (the wqkv DMA is split per-tensor and interleaved with the input
        # chunk loads below so the K weights + first K tokens arrive first)
        wqkv_sb = singles.tile([P, 3, KC, P], BF16)
        wqt_sb = wqkv_sb[:, 0]
        wkt_sb = wqkv_sb[:, 1]
        wvt_sb = wqkv_sb[:, 2]
        if with_bias:
            bq_sb = singles.tile([P, 1], F32)
            nc.sync.dma_start(out=bq_sb, in_=bq_d)
            bk_sb = singles.tile([P, 1], F32)
            nc.sync.dma_start(out=bk_sb, in_=bk_d)
            bv_sb = singles.tile([P, P], F32)
            nc.gpsimd.dma_start(
                out=bv_sb,
                in_=bass.AP(tensor=bv_d.tensor, offset=0, ap=[[0, P], [1, P]]),
            )

        # identity for the PE transposes in the final tail
        ident = singles.tile([P, P], BF16)
        masks.make_identity(nc, ident[:, :])
        if SCHRAUD_KTS:
            expc_sb = singles.tile([P, 1], F32)
            nc.gpsimd.memset(expc_sb, EXPC)

        # ---- chunked input loads (512-token slices) ----
        qt_sb = singles.tile([P, KC, T], BF16)
        kt_sb = singles.tile([P, KC, T], BF16)
        vt_sb = singles.tile([P, KC, T], BF16)
        # qb0 only needs QT chunk 0; all of KT/VT gate qb0's PV chain,
        # so load those first and defer QT chunks 1-3.
        # First K slice is only 128 tokens (one k-tile) so the first
        # QK->exp fires as soon as possible.
        ktr = kt_d.rearrange("(c p) t -> p c t", p=P)
        qtr = qt_d.rearrange("(c p) t -> p c t", p=P)
        vtr = vt_d.rearrange("(c p) t -> p c t", p=P)
        nc.sync.dma_start(out=kt_sb[:, :, 0:P], in_=ktr[:, :, 0:P])
        nc.sync.dma_start(out=wqkv_sb[:, 0:2], in_=wqkv_d[:, 0:2])
        nc.sync.dma_start(out=qt_sb[:, :, 0 : T // NTC // 2], in_=qtr[:, :, 0 : T // NTC // 2])
        nc.sync.dma_start(
            out=qt_sb[:, :, T // NTC // 2 : T // NTC],
            in_=qtr[:, :, T // NTC // 2 : T // NTC],
        )
        nc.sync.dma_start(out=kt_sb[:, :, P : T // NTC], in_=ktr[:, :, P : T // NTC])
        nc.sync.dma_start(out=wqkv_sb[:, 2], in_=wqkv_d[:, 2])
        # KT chunk c+1 is prefetched ahead of QT/VT chunk c: K gates the
        # QK->exp critical path while V only feeds the lagging PV chain.
        for c in range(1, NTC):
            sl = bass.ts(c, T // NTC)
            nc.sync.dma_start(out=kt_sb[:, :, sl], in_=ktr[:, :, sl])
            slp = bass.ts(c - 1, T // NTC)
            if c >= 2:
                nc.sync.dma_start(out=qt_sb[:, :, slp], in_=qtr[:, :, slp])
            nc.sync.dma_start(out=vt_sb[:, :, slp], in_=vtr[:, :, slp])
        slz = bass.ts(NTC - 1, T // NTC)
        nc.sync.dma_start(out=qt_sb[:, :, slz], in_=qtr[:, :, slz])
        nc.sync.dma_start(out=vt_sb[:, :, slz], in_=vtr[:, :, slz])
        wot_sb = singles.tile([P, D], BF16)
        nc.sync.dma_start(out=wot_sb, in_=wot_d)

        # ---- projections ----
        qT = qk_pool.tile([P, T], BF16)
        kT = qk_pool.tile([P, T], BF16)
        v_aug = [None] * NKT

        bqs = bq_sb if with_bias else None
        bks = bk_sb if with_bias else None

        def emit_qk_proj(dst, src_sb, w_sb, b_sb, c, lo=None, cs=None):
            if cs is None:
                cs = T // NTC
            sl = bass.ds(c * (T // NTC) if lo is None else lo, cs)
            psq = ps_mi.tile([P, QB], F32, tag="mi")
            for kc in range(KC):
                nc.tensor.matmul(
                    psq[:, 0:cs],
                    w_sb[:, kc, :],
                    src_sb[:, kc, sl],
                    start=(kc == 0),
                    stop=(kc == KC - 1),
                )
            nc.vector.tensor_copy(dst[:, sl], psq[:, 0:cs])
            if b_sb is not None:
                nc.vector.tensor_add(
                    dst[:, sl], dst[:, sl], b_sb[:, :].broadcast_to([P, cs])
                )

        def emit_v_proj(kt):
            psv = ps_mi.tile([P, P], F32, tag="mi")
            for kc in range(KC):
                nc.tensor.matmul(
                    psv,
                    vt_sb[:, kc, bass.ts(kt, P)],
                    wvt_sb[:, kc, :],
                    start=(kc == 0),
                    stop=(kc == KC - 1),
                )
            va = v_pool.tile([P, 2, DK + 1], BF16, tag="vaug")
            nc.vector.tensor_copy(
                va[:, :, 0:DK], psv[:, :].rearrange("p (h f) -> p h f", h=2)
            )
            if with_bias:
                nc.vector.tensor_add(
                    va[:, :, 0:DK],
                    va[:, :, 0:DK],
                    bv_sb[:, :].rearrange("p (h f) -> p h f", h=2),
                )
            nc.gpsimd.memset(va[:, :, DK : DK + 1], 1.0)
            v_aug[kt] = va

        # ---- attention ----
        # PSUM rule: a matmul accumulation `start` clears the has_written
        # bits of the WHOLE bank, so the 8 interleaved pv accumulation
        # groups (2 heads x 4 q-tiles, one bank per head) cannot each issue
        # their own start.  Instead, per bank only the FIRST kt0 matmul
        # (qs0) starts -- its whole-bank bit-clear is exactly right at kt0:
        # the other q-tiles' kt0 writes then overwrite-and-set their own
        # regions (flags=0 semantics), and every later kt accumulates.
        def emit_pv(pv, kt, ex):
            # transposed PV: out[q, dk+1] accumulates over key tiles
            for h in range(2):
                for qs in range(QSUB):
                    nc.tensor.matmul(
                        pv[:, h, qs, 0 : DK + 1],
                        ex[:, h, bass.ts(qs, P)],
                        v_aug[kt][:, h, :],
                        start=(kt == 0 and qs == 0),
                        stop=(kt == NKT - 1),
                        skip_group_check=True,
                    )

        def emit_attn_group(qb, pv, kts, pending, lag=True):
            # software-pipelined: PV(kt) is emitted AFTER QK/exp(kt+1) so the
            # exp engines are never starved of fresh scores.
            for kt in kts:
                sc = ps_s.tile([P, 2, QB], F32, tag="sc")
                for h in range(2):
                    nc.tensor.matmul(
                        sc[:, h, :],
                        kT[DK * h : DK * (h + 1), bass.ts(kt, P)],
                        qT[DK * h : DK * (h + 1), bass.ts(qb, QB)],
                        start=True,
                        stop=True,
                    )
                if (qb, kt) in SCHRAUD_KTS:
                    exi = exp_pool.tile([P, 2, QB], I16, tag="exps")
                    nc.vector.scalar_tensor_tensor(
                        out=exi,
                        in0=sc,
                        scalar=EXPF,
                        in1=expc_sb[:, :].broadcast_to([P, 2, QB]),
                        op0=mybir.AluOpType.mult,
                        op1=mybir.AluOpType.add,
                    )
                    ex = exi.bitcast(BF16)
                else:
                    ex = exp_pool.tile([P, 2, QB], BF16, tag="exps")
                    nc.scalar.activation(
                        out=ex,
                        in_=sc,
                        func=mybir.ActivationFunctionType.Exp,
                        scale=inv_sqrt_dk,
                    )
                if len(pending) >= 2:
                    emit_pv(*pending.pop(0))
                if lag:
                    pending.append((pv, kt, ex))
                else:
                    emit_pv(pv, kt, ex)

        def emit_tail_front(qb, pv, last):
            # normalize att[q, h, f] = pv[q, h, f] / pv[q, h, 64] and kick
            # off the att -> [feat, q] transposes (DMA XBAR mid-kernel, PE
            # transpose on the drain-critical final block).  On the final
            # block the normalize/copy work splits DVE/ACT to shorten the
            # serial drain (ACT does it as a Copy-activation with a
            # per-partition scale operand).
            rall = rden_pool.tile([P, 2, QSUB], F32, tag="rden")
            nc.vector.reciprocal(rall, pv[:, :, :, DK])
            atts = []
            for qs in range(QSUB):
                att = att_pool.tile([P, 2, DK], BF16, tag="att")
                for h in range(2):
                    if last and qs % 2 == 1:
                        nc.scalar.activation(
                            out=att[:, h, :],
                            in_=pv[:, h, qs, 0:DK],
                            func=mybir.ActivationFunctionType.Copy,
                            scale=rall[:, h, qs : qs + 1],
                        )
                    else:
                        nc.vector.tensor_mul(
                            att[:, h, :],
                            pv[:, h, qs, 0:DK],
                            rall[:, h, qs : qs + 1].broadcast_to([P, DK]),
                        )
                # PE transpose through a scores-pool psum slot: a pure
                # engine-to-engine chain (mul -> transpose -> copy -> Wo)
                # with no DMA latency for the scheduler to mispredict.
                # transposes go through the mi ring mid-kernel; on the final
                # block the scores pool is idle, so use it there to decouple
                # the transpose chain from the Wo/out-copy ring.
                attT = attT_pool.tile([P, P], BF16, tag="attT")
                if last:
                    trp = ps_s.tile([P, P], BF16, tag="sc")
                else:
                    trp = ps_mi.tile([P, P], BF16, tag="mi")
                nc.tensor.matmul(
                    trp,
                    att[:, :, :].rearrange("p h f -> p (h f)"),
                    ident,
                    is_transpose=True,
                )
                if last and qs % 2 == 1:
                    nc.scalar.copy(attT, trp)
                else:
                    nc.vector.tensor_copy(attT, trp)
                atts.append(attT)
            return atts

        def emit_tail_back_qs(qb, atts, qs, last):
            attT = atts[qs]
            pso = ps_mi.tile([P, D], F32, tag="mi")
            nc.tensor.matmul(pso, attT, wot_sb, start=True, stop=True)
            out_sb = out_pool.tile([P, D], F32, tag="outs")
            if last and qs % 2 == 1:
                nc.scalar.copy(out_sb, pso)
            else:
                nc.vector.tensor_copy(out_sb, pso)
            # Mid-kernel out DMAs all go on the Pool SWDGE queue: their sem
            # waits (on late DVE copies) must NOT block SP.SEQ, which next
            # issues the following tail's XBAR transposes (head-of-line).
            # The final block uses SP+ACT (lowest issue latency) instead.
            if last:
                eng = nc.sync if qs % 2 == 0 else nc.scalar
            else:
                eng = nc.gpsimd
            eng.dma_start(out=out_d[bass.ds(qb * QB + qs * P, P), :], in_=out_sb)

        # qb0 is interleaved with the per-chunk projections so the exp
        # engines (the bottleneck) start as early as possible.
        pending = []
        pv0 = ps_pv.tile([P, 2, QSUB, P], F32, tag="pv")
        emit_qk_proj(kT, kt_sb, wkt_sb, bks, 0, lo=0, cs=P)
        emit_qk_proj(qT, qt_sb, wqt_sb, bqs, 0, lo=0, cs=T // NTC // 2)
        # kt0 is processed in two half-query pieces so the first softmax exp
        # fires as soon as only HALF of qb0's queries are projected.
        HQ = QB // 2
        sc0 = ps_s.tile([P, 2, QB], F32, tag="sc")
        ex0 = exp_pool.tile([P, 2, QB], BF16, tag="exps")
        for h in range(2):
            nc.tensor.matmul(
                sc0[:, h, 0:HQ],
                kT[DK * h : DK * (h + 1), 0:P],
                qT[DK * h : DK * (h + 1), 0:HQ],
                start=True,
                stop=True,
            )
        nc.scalar.activation(
            out=ex0[:, :, 0:HQ],
            in_=sc0[:, :, 0:HQ],
            func=mybir.ActivationFunctionType.Exp,
            scale=inv_sqrt_dk,
        )
        emit_qk_proj(qT, qt_sb, wqt_sb, bqs, 0, lo=HQ, cs=T // NTC // 2)
        for h in range(2):
            nc.tensor.matmul(
                sc0[:, h, HQ:QB],
                kT[DK * h : DK * (h + 1), 0:P],
                qT[DK * h : DK * (h + 1), HQ:QB],
                start=True,
                stop=True,
            )
        nc.scalar.activation(
            out=ex0[:, :, HQ:QB],
            in_=sc0[:, :, HQ:QB],
            func=mybir.ActivationFunctionType.Exp,
            scale=inv_sqrt_dk,
        )
        pending.append((pv0, 0, ex0))
        emit_qk_proj(kT, kt_sb, wkt_sb, bks, 0, lo=P, cs=T // NTC - P)
        for c in range(NTC):
            # next chunk's k/q projections are emitted BEFORE this chunk's
            # v-projections so their psum allocations are FIFO-ahead of the
            # (VT-gated) psv tiles in the shared misc pool.
            if c + 1 < NTC:
                emit_qk_proj(kT, kt_sb, wkt_sb, bks, c + 1)
            if c == 0:
                for kt in range(4):
                    emit_v_proj(kt)
            lo = 1 if c == 0 else 4 * c
            emit_attn_group(0, pv0, range(lo, 4 * (c + 1)), pending)
            if c + 1 < NTC:
                for kt in range(4 * (c + 1), 4 * (c + 2)):
                    emit_v_proj(kt)
            if c + 1 < NTC:
                emit_qk_proj(qT, qt_sb, wqt_sb, bqs, c + 1)
        # Each qb's pending PVs are FLUSHED at its end (kt15's exp stays on
        # ACT so the flush only waits one activation), and its tail front
        # (normalize + transposes) runs immediately at the boundary.  That
        # puts the critical DVE muls AHEAD of the next qb's copies in the
        # in-order DVE queue, and gives the XBAR transposes a full qb of
        # latency slack before the Wo chunks (spread at groups 4/7/10/13 of
        # the next qb, alternating DVE copies with DVE exp offloads) need
        # them.
        while pending:
            emit_pv(*pending.pop(0))
        prev = (0, pv0)
        atts = emit_tail_front(*prev, last=False)
        for qb in range(1, NQB):
            pv = ps_pv.tile([P, 2, QSUB, P], F32, tag="pv")
            last = qb == NQB - 1
            emit_attn_group(qb, pv, range(0, 4), pending)
            emit_tail_back_qs(prev[0], atts, 0, last=False)
            emit_attn_group(qb, pv, range(4, 7), pending)
            emit_tail_back_qs(prev[0], atts, 1, last=False)
            emit_attn_group(qb, pv, range(7, 10), pending)
            emit_tail_back_qs(prev[0], atts, 2, last=False)
            emit_attn_group(qb, pv, range(10, 13), pending)
            emit_tail_back_qs(prev[0], atts, 3, last=False)
            emit_attn_group(qb, pv, range(13, NKT), pending)
            while pending:
                emit_pv(*pending.pop(0))
            prev = (qb, pv)
            atts = emit_tail_front(*prev, last=last)
        for qs in range(QSUB):
            emit_tail_back_qs(prev[0], atts, qs, last=True)

    nc.compile()
    return nc


_NC_CACHE = {}


def _get_nc(with_bias):
    if with_bias not in _NC_CACHE:
        _NC_CACHE[with_bias] = _build_bass(with_bias)
    return _NC_CACHE[with_bias]


def _prep_in_maps(Q, K, V, Wq, bq, Wk, bk, Wv, bv, Wo, bo, with_bias):
    bf = ml_dtypes.bfloat16
    f32 = np.float32
    qkvT = []  # per batch: transposed bf16 [D, T]
    for X in (Q, K, V):
        qkvT.append([np.ascontiguousarray(X[b].T.astype(bf)) for b in range(B)])
    woT = np.ascontiguousarray(Wo.T.astype(bf))  # [D feat, D out]

    def swz(w_rows):  # [P, D] slice of W -> transposed+partition-major [P, KC, P]
        return np.ascontiguousarray(
            w_rows.T.astype(bf).reshape(KC, P, P).transpose(1, 0, 2)
        )

    in_maps = []
    for c in range(N_CORES):
        b, p = divmod(c, 4)
        rows = slice(P * p, P * (p + 1))
        m = {
            "qt": qkvT[0][b],
            "ktin": qkvT[1][b],
            "vt": qkvT[2][b],
            "wqkv": np.ascontiguousarray(
                np.stack([swz(Wq[rows]), swz(Wk[rows]), swz(Wv[rows])], axis=1)
            ),
            "wot": np.ascontiguousarray(woT[rows]),
        }
        if with_bias:
            m["bq"] = np.ascontiguousarray(bq[rows].astype(f32).reshape(P, 1))
            m["bk"] = np.ascontiguousarray(bk[rows].astype(f32).reshape(P, 1))
            m["bv"] = np.ascontiguousarray(bv[rows].astype(f32).reshape(1, P))
        in_maps.append(m)
    return in_maps


def kernel(Q, K, V, Wq, bq, Wk, bk, Wv, bv, Wo, bo, _return_raw=False):
    # accept jax arrays / lists transparently
    Q, K, V = np.asarray(Q), np.asarray(K), np.asarray(V)
    Wq, Wk, Wv, Wo = (np.asarray(x) for x in (Wq, Wk, Wv, Wo))
    bq, bk, bv, bo = (np.asarray(x) for x in (bq, bk, bv, bo))
    with_bias = bool(np.any(bq) or np.any(bk) or np.any(bv))
    nc = _get_nc(with_bias)
    in_maps = _prep_in_maps(Q, K, V, Wq, bq, Wk, bk, Wv, bv, Wo, bo, with_bias)
    try:
        res = run_bass_kernel_spmd(
            nc,
            in_maps,
            core_ids=list(range(N_CORES)),
            trace=os.environ.get("KERNEL_TRACE", "0") == "1",
        )
    except ModuleNotFoundError:
        # BASS_TRACE was requested but this axon build lacks the NTFF
        # profile hook (antenv.axon_hooks) -- rerun with tracing disabled.
        os.environ["BASS_NEVER_TRACE"] = "1"
        res = run_bass_kernel_spmd(
            nc, in_maps, core_ids=list(range(N_CORES)), trace=False
        )
    parts = [r["outp"] for r in res.results]
    out = np.empty((B, T, D), np.float32)
    for b in range(B):
        out[b] = parts[4 * b]
        for p in range(1, 4):
            out[b] += parts[4 * b + p]
        out[b] += bo.astype(np.float32)
    if _return_raw:
        return out, res
    return out
